# revision 12
# baseline (speedup 1.0000x reference)
"""Multi-head attention with random-synthesizer blend + mask, on 8 Trainium2
NeuronCores.  Sharding: data-parallel over batch (B=8 -> one core each).

Key algebraic restructure (v2+): the softmax exponential is factored as
    exp(alpha*scores + (1-alpha)*syn) = exp(alpha*scores) * exp((1-alpha)*syn)
so the synthesizer + mask enter as one precomputed fp16 multiplier
EMT[h,k,q] = exp((1-alpha)*syn[h,q,k]) * mask[q,k], built on the host and
DMA'd as plain contiguous [128, 2, S] pair tiles.  This removes the
on-device syn transpose DMAs, the mask prep phase, and the per-tile PE
identity-matmul syn add of the original kernel.

v3: everything fp16 on the PE (fp8 fails the 2e-2 gate: for zero-mean
random sums quantization noise passes through ~1:1, so fp8's ~4 % per-cast
noise lands ~4 % on the output).  Phases are strictly separated so the PE
queue runs back-to-back and earns its 2.4 GHz p-state: v/q/k projections,
then 16 heads of attention, then the output projection.

Per-core attention, per head: scores^T = k^T q (PSUM, c1=alpha/sqrt(64)
folded into q's evacuation), p = exp(.) on ACT, pm = p * EMT on DVE (every
4th tile on Pool), and pav += [v|ones]^T pm on PE accumulates both the
unnormalized output and the softmax sums ([v|ones] column interleave).
Normalization is deferred: per head pair, swap the sums halves with a tiny
SBUF-SBUF DMA, reciprocal_approx_fast (fp32), multiply into fp16 otn.

Engine split: ACT = exp + final evac; DVE = EMT mult, projection/pav
drains, normalize; Pool = 1/4 of EMT mults + half the DMA issue; SP = the
other DMA issue half.
"""

import math
import sys

sys.path.insert(0, "/opt/trn_rl_repo")

import numpy as np

import concourse.tile as tile
import concourse.mybir as mybir
from concourse import bacc
from concourse.bass_utils import run_bass_kernel_spmd

B, S, D, H = 8, 1024, 1024, 16
HD = D // H  # 64
N_CORES = 8
P = 128
SC = S // P  # 8
DC = D // P  # 8
HP = H // 2  # 8 head pairs

f32 = mybir.dt.float32
fp16 = mybir.dt.float16
AF = mybir.ActivationFunctionType
OP = mybir.AluOpType

# test harness knobs (the grading entry point `kernel` leaves these alone)
TRACE = False
TRACE_TMPDIR = None
LAST_RESULTS = None

_CACHE = {}


def _emit(nc, tc, dram, c1):
    with (
        tc.tile_pool(name="pers", bufs=1) as pers,
        tc.tile_pool(name="psum", bufs=1, space="PSUM") as psum,
    ):
        bq_sb = pers.tile([P, DC], f32, tag="bq")
        bk_sb = pers.tile([P, DC], f32, tag="bk")
        bo_sb = pers.tile([P, DC], f32, tag="bo")
        wo_t = [pers.tile([P, D], fp16, tag=f"wo{i}", name=f"wo{i}") for i in range(DC)]

        # ---- persistent activations --------------------------------------
        qT = [pers.tile([P, S], fp16, tag=f"qT{i}", name=f"qT{i}") for i in range(DC)]
        kT = [pers.tile([P, S], fp16, tag=f"kT{i}", name=f"kT{i}") for i in range(DC)]
        # per s-chunk: 8 head-pair blocks of 256 cols; head h=2j+i occupies
        # [i*128,(i+1)*128) of block j as [v|ones] (even) / [ones|v] (odd)
        v_sb = [pers.tile([P, SC, 256], fp16, tag=f"v{i}", name=f"v{i}") for i in range(SC)]
        otn = [pers.tile([P, S], fp16, tag=f"otn{i}", name=f"otn{i}") for i in range(DC)]

        def load8(name, pool, tag, eng=None):
            ts_ = []
            for j in range(DC):
                t = pool.tile([P, S], fp16, tag=f"{tag}{j}", bufs=1, name=f"{tag}{j}")
                e = eng if eng is not None else (nc.sync if j % 2 == 0 else nc.gpsimd)
                e.dma_start(out=t[:], in_=dram[name][j * P:(j + 1) * P, :])
                ts_.append(t)
            return ts_

        # ================= phase 1: projections ==========================
        with tc.tile_pool(name="ph1", bufs=1) as pv:
            xv, wv = [], []
            qs = [nc.sync, nc.gpsimd, nc.scalar]
            for j in range(DC):
                t = pv.tile([P, S], fp16, tag=f"xv{j}", bufs=1, name=f"xv{j}")
                qs[(2 * j) % 3].dma_start(out=t[:], in_=dram["xv"][j * P:(j + 1) * P, :])
                xv.append(t)
                t = pv.tile([P, S], fp16, tag=f"wv{j}", bufs=1, name=f"wv{j}")
                qs[(2 * j + 1) % 3].dma_start(out=t[:], in_=dram["wv"][j * P:(j + 1) * P, :])
                wv.append(t)
            # biases + Wo after the hot v-proj inputs (needed much later)
            nc.sync.dma_start(out=bq_sb[:], in_=dram["bqc1"].rearrange("(c p) -> p c", p=P))
            nc.sync.dma_start(out=bk_sb[:], in_=dram["bk"].rearrange("(c p) -> p c", p=P))
            nc.sync.dma_start(out=bo_sb[:], in_=dram["boeff"].rearrange("(c p) -> p c", p=P))
            for i in range(DC):
                nc.gpsimd.dma_start(out=wo_t[i][:], in_=dram["wo"][i * P:(i + 1) * P, :])
            for sc in range(SC):
                nc.vector.memset(v_sb[sc][:], 1.0)
            for sc in range(SC):
                ps = psum.tile([P, 8, P], f32, tag="mm", bufs=3, name=f"psv{sc}")
                for dq in range(2):
                    for di in range(DC):
                        nc.tensor.matmul(
                            ps[:, 4 * dq:4 * dq + 4, :],
                            xv[di][:, sc * P:(sc + 1) * P],
                            wv[di][:, dq * 512:(dq + 1) * 512],
                            start=(di == 0), stop=(di == DC - 1),
                        )
                # even heads' v: psum cols [0:64) of each 128-block
                nc.vector.tensor_copy(out=v_sb[sc][:, :, 0:64], in_=ps[:, :, 0:64])
                nc.vector.tensor_copy(out=v_sb[sc][:, :, 192:256], in_=ps[:, :, 64:128])

            xq = load8("xq", pv, "xq")
            wq = load8("wq", pv, "wq")
            xk = load8("xk", pv, "xk")
            wk = load8("wk", pv, "wk")
            for nm, dst, xs, ws_, scale, bias in (
                ("q", qT, xq, wq, c1, bq_sb),
                ("k", kT, xk, wk, 1.0, bk_sb),
            ):
                for do in range(DC):
                    ps = psum.tile([P, 8, P], f32, tag="mm", bufs=3,
                                   name=f"psp{nm}{do}")
                    for sq in range(2):
                        for di in range(DC):
                            nc.tensor.matmul(
                                ps[:, 4 * sq:4 * sq + 4, :],
                                ws_[di][:, do * P:(do + 1) * P],
                                xs[di][:, sq * 512:(sq + 1) * 512],
                                start=(di == 0), stop=(di == DC - 1),
                            )
                    nc.vector.tensor_scalar(
                        out=dst[do][:], in0=ps[:], scalar1=float(scale),
                        scalar2=bias[:, do:do + 1], op0=OP.mult, op1=OP.add,
                    )

        # ================= phase 2: attention ============================
        # Software-pipelined: scores/exp/mult stream one kc per step; attnV
        # trails by one 4-kc block and is emitted sq-major so consecutive
        # matmuls extend the same PSUM accumulation chain (hides the PE's
        # per-group SBUF access latency).  The normalize chain is deferred
        # and split so it never sits in front of the next head's DVE work.
        with tc.tile_pool(name="attn", bufs=1) as ap:
            praw = [None] * H
            pav_t = {}
            pm_t = {}
            emt_t = {}
            pend = []

            def emit_recip(hp):
                he, ho = praw[2 * hp], praw[2 * hp + 1]
                rectmp = ap.tile([P, S], f32, tag="rtm", bufs=2, name=f"rt{hp}")
                nc.sync.dma_start(out=rectmp[0:HD, :], in_=he[HD:P, :])
                nc.sync.dma_start(out=rectmp[HD:P, :], in_=ho[0:HD, :])
                rec = ap.tile([P, S], f32, tag="rec", bufs=2, name=f"rc{hp}")
                nc.vector.reciprocal_approx_fast(out=rec[:], in_=rectmp[:])
                return rec

            def emit_norm(hp, rec):
                # otn rows [0:64)=head 2hp dims, [64:128)=head 2hp+1 dims
                he, ho = praw[2 * hp], praw[2 * hp + 1]
                nc.vector.tensor_tensor(
                    out=otn[hp][0:HD, :], in0=he[0:HD, :], in1=rec[0:HD, :],
                    op=OP.mult,
                )
                nc.vector.tensor_tensor(
                    out=otn[hp][HD:P, :], in0=ho[HD:P, :], in1=rec[HD:P, :],
                    op=OP.mult,
                )

            NIT = H * SC
            for step in range(NIT + 8):
                while pend and pend[0][0] <= step:
                    pend.pop(0)[1]()
                if step < NIT:
                    h, kc = step // SC, step % SC
                    hp, hodd = h // 2, h % 2
                    if kc == 0:
                        pav_t[h] = [psum.tile([P, 512], f32, tag="pav", bufs=2,
                                              name=f"pav{h}_{i}") for i in range(2)]
                    if kc % 2 == 0:
                        t = kc // 2
                        eng = nc.sync if t % 2 == 0 else nc.gpsimd
                        emt2 = ap.tile([P, 2, S], fp16, tag="emt", bufs=6,
                                       name=f"emt{h}_{t}")
                        eng.dma_start(out=emt2[:], in_=dram["emt"][h, t])
                        emt_t[h] = emt2
                    ps = psum.tile([P, 8, P], f32, tag="mm", bufs=3,
                                   name=f"pss{h}_{kc}")
                    for sq in range(2):
                        nc.tensor.matmul(
                            ps[:, 4 * sq:4 * sq + 4, :],
                            kT[hp][hodd * HD:(hodd + 1) * HD, kc * P:(kc + 1) * P],
                            qT[hp][hodd * HD:(hodd + 1) * HD, sq * 512:(sq + 1) * 512],
                            start=True, stop=True,
                        )
                    p = ap.tile([P, S], fp16, tag="p", bufs=3, name="p")
                    nc.scalar.activation(out=p[:], in_=ps[:], func=AF.Exp)
                    pm = ap.tile([P, S], fp16, tag="pm", bufs=9, name="pm")
                    meng = nc.gpsimd if kc % 4 == 3 else nc.vector
                    meng.tensor_tensor(
                        out=pm[:], in0=p[:], in1=emt_t[h][:, kc % 2, :], op=OP.mult,
                    )
                    pm_t[(h, kc)] = pm
                b = step - 4
                if 0 <= b < NIT and b % 4 == 3:
                    h, kc3 = b // SC, b % SC
                    hp, hodd = h // 2, h % 2
                    base = kc3 - 3
                    for sq in range(2):
                        for j in range(4):
                            kc = base + j
                            nc.tensor.matmul(
                                pav_t[h][sq][:],
                                v_sb[kc][:, hp, hodd * P:(hodd + 1) * P],
                                pm_t[(h, kc)][:, sq * 512:(sq + 1) * 512],
                                start=(kc == 0), stop=(kc == SC - 1),
                            )
                    for j in range(4):
                        del pm_t[(h, base + j)]
                    if kc3 != SC - 1:
                        continue
                    # praw[h] rows = [out;sums] (even head) / [sums;out] (odd)
                    pr = ap.tile([P, S], f32, tag="praw", bufs=4, name=f"pr{h}")
                    praw[h] = pr
                    for sq in range(2):
                        nc.vector.tensor_copy(
                            out=pr[:, sq * 512:(sq + 1) * 512],
                            in_=pav_t[h][sq][:],
                        )
                    if hodd == 1:
                        hp_ = hp
                        box = {}
                        pend.append((step + 2, lambda hp=hp_, box=box: box.__setitem__('rec', emit_recip(hp))))
                        pend.append((step + 4, lambda hp=hp_, box=box: emit_norm(hp, box['rec'])))
            while pend:
                pend.pop(0)[1]()

            # ================= phase 3: output projection ================
            for dd in range(DC):
                ps = psum.tile([P, 8, P], f32, tag="mm", bufs=3,
                               name=f"pso{dd}")
                for sq in range(2):
                    for ci in range(DC):
                        nc.tensor.matmul(
                            ps[:, 4 * sq:4 * sq + 4, :],
                            wo_t[ci][:, dd * P:(dd + 1) * P],
                            otn[ci][:, sq * 512:(sq + 1) * 512],
                            start=(ci == 0), stop=(ci == DC - 1),
                        )
                osb = ap.tile([P, S], fp16, tag="osb", bufs=3, name=f"osb{dd}")
                if dd % 2 == 0:
                    nc.scalar.activation(
                        out=osb[:], in_=ps[:], func=AF.Identity,
                        bias=bo_sb[:, dd:dd + 1], scale=1.0,
                    )
                else:
                    nc.vector.tensor_scalar(
                        out=osb[:], in0=ps[:], scalar1=1.0,
                        scalar2=bo_sb[:, dd:dd + 1], op0=OP.mult, op1=OP.add,
                    )
                nc.gpsimd.dma_start(
                    out=dram["outT"][dd * P:(dd + 1) * P, :], in_=osb[:],
                )


def _build(c1):
    nc = bacc.Bacc("TRN2", debug=False)
    dram = {
        "xq": nc.declare_dram_parameter("xq", [D, S], fp16, isOutput=False),
        "xk": nc.declare_dram_parameter("xk", [D, S], fp16, isOutput=False),
        "xv": nc.declare_dram_parameter("xv", [D, S], fp16, isOutput=False),
        "wq": nc.declare_dram_parameter("wq", [D, D], fp16, isOutput=False),
        "wk": nc.declare_dram_parameter("wk", [D, D], fp16, isOutput=False),
        "wv": nc.declare_dram_parameter("wv", [D, D], fp16, isOutput=False),
        "wo": nc.declare_dram_parameter("wo", [D, D], fp16, isOutput=False),
        "bqc1": nc.declare_dram_parameter("bqc1", [D], f32, isOutput=False),
        "bk": nc.declare_dram_parameter("bk", [D], f32, isOutput=False),
        "boeff": nc.declare_dram_parameter("boeff", [D], f32, isOutput=False),
        # [h, kc-pair t, partition p, slot i, q]: k position = 256t+128i+p
        "emt": nc.declare_dram_parameter("emt", [H, 4, P, 2, S], fp16, isOutput=False),
        "outT": nc.declare_dram_parameter("outT", [D, S], fp16, isOutput=True),
    }
    with tile.TileContext(nc) as tc:
        _emit(nc, tc, dram, c1)
    nc.compile()
    return nc


def kernel(**inputs):
    global LAST_RESULTS
    q = np.asarray(inputs["query"], np.float32)
    k = np.asarray(inputs["key"], np.float32)
    v = np.asarray(inputs["value"], np.float32)
    msk = np.asarray(inputs["mask"], np.int32)
    ws = {nm: np.asarray(inputs["W" + nm], np.float32) for nm in "qkvo"}
    bs = {nm: np.asarray(inputs["b" + nm], np.float32) for nm in "qkvo"}
    alpha = float(1.0 / (1.0 + math.exp(-float(np.asarray(inputs["alpha_param"]).ravel()[0]))))
    c1 = alpha / math.sqrt(HD)
    c2 = 1.0 - alpha

    w16 = {nm: ws[nm].astype(np.float16) for nm in "qkvo"}
    boeff = (bs["v"].astype(np.float64) @ ws["o"].astype(np.float64)
             + bs["o"]).astype(np.float32)
    bqc1 = (bs["q"] * c1).astype(np.float32)

    # shared syn part of the softmax multiplier, pre-transposed to [h, k, q]
    syn = np.asarray(inputs["syn_scores"], np.float32)[:, :S, :S]
    et16 = np.exp(c2 * syn.transpose(0, 2, 1)).astype(np.float16)

    key_ = (round(c1, 12),)
    if key_ not in _CACHE:
        _CACHE[key_] = _build(c1)
    nc = _CACHE[key_]

    in_maps = []
    for b in range(B):
        mt = (msk[b].T != 0)
        emt = np.where(mt[None, :, :], et16, np.float16(0.0))  # [H, k, q]
        # [H, S, S] -> [H, 4, P, 2, S]: k = 256t + 128i + p
        emt = np.ascontiguousarray(
            emt.reshape(H, 4, 2, P, S).transpose(0, 1, 3, 2, 4))
        in_maps.append({
            "xq": q[b].T.astype(np.float16),
            "xk": k[b].T.astype(np.float16),
            "xv": v[b].T.astype(np.float16),
            "wq": w16["q"], "wk": w16["k"], "wv": w16["v"], "wo": w16["o"],
            "bqc1": bqc1, "bk": bs["k"], "boeff": boeff,
            "emt": emt,
        })

    kwargs = {}
    if TRACE:
        kwargs["trace"] = True
        if TRACE_TMPDIR:
            kwargs["tmpdir"] = TRACE_TMPDIR
    res = run_bass_kernel_spmd(nc, in_maps, core_ids=list(range(N_CORES)), **kwargs)
    LAST_RESULTS = res
    return np.stack(
        [res.results[b]["outT"].astype(np.float32).T for b in range(B)], axis=0
    )
